# revision 24
# baseline (speedup 1.0000x reference)
"""APPNP forward on 8 TRN2 NeuronCores (Bass/Tile).

Math (reference):
    h0  = relu(x @ W1)                       [N, H]
    x_cl = h0; repeat K: x_cl = 0.9*(adj @ x_cl) + 0.1*h0
    out = (log_softmax(x_cl @ W2, -1), x_cl)

Distribution: row-shard adj/x across 8 cores (SHARD = N/8 rows each).
Per step each core computes its block-row adj_i @ x_cl with x_cl
replicated via AllGather.

Kernel strategy (per core):
  - adj_i is transposed+quantized ONCE on-chip (fused into step 1):
    f32 -> ACT scale by 4096 -> bf16 -> PE transpose -> fp8e4 (stored
    as adjT8 [N, SHARD] in DRAM).  Row-stochastic averaging makes the
    fp8 quantization error negligible (measured ~4e-4 end-to-end).
  - x_cl is carried in an fp8e4 hi/lo split (~9e-4 end-to-end): the
    stationary operand packs [hi | lo] -> psum [128, m] accumulates
    (adj@x)^T in hi/lo halves; epilogue recombines with 0.9/4096 scale
    and adds 0.1*h0.
  - Per step: 256 DoubleRow fp8 matmuls (64 c-tile pairs x 4 m-chunks
    of 512); the moving operand is the fp8 adjT stream in a
    partition-major DRAM layout (16KB-contiguous runs, 32MB/core/step).
  - x_cl^T [64, SHARD] -> hi/lo fp8 -> PE transpose (stride-2 psum) ->
    [SHARD, 128] fp8 pair tiles -> AllGather -> next step's stationary.
  - Final step skips the AllGather and computes the log_softmax head
    locally.
"""

import numpy as np

import concourse.bass as bass
import concourse.bacc as bacc
import concourse.mybir as mybir
import concourse.tile as tile
from concourse.bass_utils import run_bass_kernel_spmd

# Problem constants (hardcoded; the grading harness supplies matching inputs)
N, NFEAT, NHID, NCLASS = 16384, 512, 64, 40
K, ALPHA = 10, 0.1
NCORES = 8

SC = 4096.0              # adj prescale so fp8e4 sees ~[0, 1) values
C0 = (1.0 - ALPHA) / SC  # epilogue scale for psum

dt = mybir.dt
f32, bf16, fp8 = dt.float32, dt.bfloat16, dt.float8e4
AF = mybir.ActivationFunctionType
OP = mybir.AluOpType


def build(n=N, nfeat=NFEAT, k_steps=K, ncores=NCORES):
    shard = n // ncores
    MT = shard // 128          # m-tiles per shard
    CT = n // 128              # c-tiles (contraction)
    MCH = min(512, shard)      # psum chunk (<= 1 bank)
    NCH = shard // MCH         # chunks per shard
    FT = nfeat // 128          # feature tiles
    GRP = 8                    # c-tiles per moving-stream DMA
    NGRP = CT // GRP
    H = NHID

    nc = bacc.Bacc(None)
    x_ext = nc.dram_tensor("x", [shard, nfeat], f32, kind="ExternalInput")
    adj_ext = nc.dram_tensor("adj", [shard, n], f32, kind="ExternalInput")
    w1_ext = nc.dram_tensor("W1", [nfeat, H], f32, kind="ExternalInput")
    w2_ext = nc.dram_tensor("W2", [H, NCLASS], f32, kind="ExternalInput")
    eye_ext = nc.inline_tensor(np.eye(128, dtype=np.float32), "eye128")
    out1_ext = nc.dram_tensor("out1", [shard, NCLASS], f32, kind="ExternalOutput")
    out2_ext = nc.dram_tensor("out2", [shard, H], f32, kind="ExternalOutput")

    with tile.TileContext(nc) as tc:
        with (
            tc.tile_pool(name="const", bufs=1) as constp,
            tc.tile_pool(name="hfam", bufs=1) as hfam,
            tc.tile_pool(name="stat", bufs=1) as statp,
            tc.tile_pool(name="mv", bufs=5) as mvp,
            tc.tile_pool(name="pend", bufs=1) as pend,
            tc.tile_pool(name="psy", bufs=1, space="PSUM") as psyp,
            tc.tile_pool(name="pstep", bufs=2, space="PSUM") as pstep,  # 1-bank small tiles
            tc.tile_pool(name="dram", bufs=2, space="DRAM") as dram,
            tc.tile_pool(name="dram1", bufs=1, space="DRAM") as dram1,
        ):
            # ---------------- constants ----------------
            eye32 = constp.tile([128, 128], f32)
            nc.sync.dma_start(eye32[:], eye_ext[:])
            eyeb = constp.tile([128, 128], bf16)
            nc.vector.tensor_copy(eyeb[:], eye32[:])
            eye8 = constp.tile([128, 128], fp8)
            nc.vector.tensor_copy(eye8[:], eye32[:])
            w1_sb = constp.tile([128, FT, H], f32)
            nc.sync.dma_start(w1_sb[:], w1_ext.rearrange("(t p) h -> p t h", p=128))
            w2_sb = constp.tile([H, NCLASS], f32)
            nc.sync.dma_start(w2_sb[:], w2_ext[:])

            adjT8 = dram1.tile([128, CT, shard], fp8)  # partition-major

            # tiny dummy AllGather to absorb first-collective warmup cost
            wu_in = dram1.tile([128, 16], bf16)
            wu_out = dram1.tile([128 * ncores, 16], bf16, addr_space="Shared")
            wu_sb = constp.tile([128, 16], bf16)
            nc.vector.tensor_copy(wu_sb[:], eye32[:, 0:16])
            nc.sync.dma_start(wu_in.opt(), wu_sb[:])
            nc.gpsimd.collective_compute(
                "AllGather", OP.bypass, replica_groups=[list(range(ncores))],
                ins=[wu_in.opt()], outs=[wu_out.opt()])

            # ---------------- phase 0: h0 = relu(x @ W1) ----------------
            h0s = hfam.tile([H, shard], f32)     # 0.1 * h0^T
            xclT = hfam.tile([H, shard], f32)    # current x_cl^T (f32)
            with tc.tile_pool(name="ph0", bufs=1) as ph0:
                HM = MT // 2 if MT >= 2 else MT  # m-tiles per half
                NHALF = MT // HM
                psh_full = psyp.tile([128, shard], f32, tag="psy")
                psh = psh_full[0:H, :]
                for half in range(NHALF):
                    x_sb = ph0.tile([128, HM, nfeat], f32, tag="x_sb")
                    nc.sync.dma_start(
                        x_sb[:],
                        x_ext[half * HM * 128:(half + 1) * HM * 128, :]
                        .rearrange("(t p) f -> p t f", p=128))
                    xT = ph0.tile([128, FT, HM * 128], f32, tag="xT")
                    for t in range(HM):
                        for ft in range(FT):
                            pxt = pstep.tile([128, 128], f32, tag="sm")
                            nc.tensor.transpose(
                                pxt[:], x_sb[:, t, ft * 128:(ft + 1) * 128],
                                eye32[:])
                            nc.vector.tensor_copy(
                                xT[:, ft, t * 128:(t + 1) * 128], pxt[:])
                    hb = half * HM * 128
                    nchh = (HM * 128) // MCH if HM * 128 >= MCH else 1
                    mchh = min(MCH, HM * 128)
                    for ft in range(FT):
                        for ch in range(nchh):
                            nc.tensor.matmul(
                                psh[:, hb + ch * mchh:hb + (ch + 1) * mchh],
                                w1_sb[:, ft, :],
                                xT[:, ft, ch * mchh:(ch + 1) * mchh],
                                start=(ft == 0), stop=(ft == FT - 1))
                nc.scalar.activation(h0s[:], psh[:], AF.Relu, scale=ALPHA)
                nc.scalar.activation(xclT[:], psh[:], AF.Relu, scale=1.0)

            # step-end: xclT f32 -> hi/lo fp8 -> transpose -> agin (no AG)
            def step_end_pre(xsrc, hiT=None, loT=None):
                if hiT is None:
                    hiT = pend.tile([H, shard], fp8, tag="hiT")
                    loT = pend.tile([H, shard], fp8, tag="loT")
                    nc.vector.tensor_copy(hiT[:], xsrc[:])
                    nc.vector.tensor_tensor(out=loT[:], in0=xsrc[:],
                                            in1=hiT[:], op=OP.subtract)
                xnp = pend.tile([128, MT, 128], fp8, tag="xnp")
                for t in range(MT):
                    pp = pstep.tile([128, 256], fp8, tag="sm")
                    hv = pp[:, 0:128].rearrange(
                        "p (m two) -> p m two", two=2)[:, :, 0]
                    lv = pp[:, 128:256].rearrange(
                        "p (m two) -> p m two", two=2)[:, :, 0]
                    nc.tensor.transpose(
                        hv, hiT[:, t * 128:(t + 1) * 128], eye8[0:H, 0:H])
                    nc.tensor.transpose(
                        lv, loT[:, t * 128:(t + 1) * 128], eye8[0:H, 0:H])
                    src_v = pp[:].rearrange(
                        "p (h m two) -> p h m two", h=2, two=2)[:, :, :, 0]
                    dst_v = xnp[:, t, :].rearrange("p (h m) -> p h m", h=2)
                    nc.vector.tensor_copy(dst_v, src_v)
                agin = dram.tile([shard, 128], fp8, tag="agin")
                nc.scalar.dma_start(
                    agin.opt().rearrange("(t p) f -> p t f", p=128), xnp[:])
                return agin, xnp

            def emit_ag(agin):
                xpf = dram.tile([n, 128], fp8, addr_space="Shared", tag="xpf")
                nc.gpsimd.collective_compute(
                    "AllGather", OP.bypass,
                    replica_groups=[list(range(ncores))],
                    ins=[agin.opt()], outs=[xpf.opt()])
                return xpf

            agin0, xnp_prev = step_end_pre(xclT)
            xpf = emit_ag(agin0)


            # ---------------- K propagation steps ----------------
            SGRP = 8                      # c-tiles per xstat DMA group
            NSG = CT // SGRP
            S_OWN = MT // SGRP            # groups covered by the local shard
            pid = nc.partition_id()
            rot = pid * S_OWN             # per-core group rotation
            PRE = min(2, NGRP)            # mov groups prefetched across the AG
            mov_pre = []                  # tiles DMA'd before the step's AG

            def emit_mov_dma(s):
                gsrc = (rot + s) & (NGRP - 1)
                mv_t = mvp.tile([128, GRP, shard], fp8, tag="mov")
                nc.sync.dma_start(
                    mv_t[:], adjT8[:, bass.ds(gsrc * GRP, GRP), :])
                return mv_t
            for k in range(1, k_steps + 1):
                psy = psyp.tile([128, shard], f32, tag="psy")
                xstat = statp.tile([128, CT, 128], fp8, tag="xstat")

                def load_xstat(sg):
                    nc.scalar.dma_start(
                        xstat[:, sg * SGRP:(sg + 1) * SGRP, :],
                        xpf.opt()[sg * SGRP * 128:(sg + 1) * SGRP * 128, :]
                        .rearrange("(t p) f -> p t f", p=128))

                if k == 1:
                    with (
                        tc.tile_pool(name="pp1", bufs=3) as pp1,
                        tc.tile_pool(name="pp1ps", bufs=2, space="PSUM") as pp1ps,
                    ):
                        for gp in range(NGRP // 2):
                            load_xstat(2 * gp)
                            load_xstat(2 * gp + 1)
                            mov_a = mvp.tile([128, GRP, shard], fp8, tag="mov")
                            mov_b = mvp.tile([128, GRP, shard], fp8, tag="mov")
                            W2G = 2 * GRP * 128
                            for mt in range(MT):
                                asl = pp1.tile([128, W2G], f32, tag="asl")
                                nc.sync.dma_start(
                                    asl[:],
                                    adj_ext[mt * 128:(mt + 1) * 128,
                                            gp * W2G:(gp + 1) * W2G])
                                abf = pp1.tile([128, W2G], bf16, tag="abf")
                                nc.scalar.activation(abf[:], asl[:], AF.Copy,
                                                     scale=SC)
                                for h2, mv_t in ((0, mov_a), (1, mov_b)):
                                    pst = pp1ps.tile([128, GRP, 128], bf16,
                                                     tag="pst")
                                    for j in range(GRP):
                                        nc.tensor.transpose(
                                            pst[:, j, :],
                                            abf[:, (h2 * GRP + j) * 128:
                                                (h2 * GRP + j + 1) * 128],
                                            eyeb[:])
                                    nc.vector.tensor_copy(
                                        mv_t[:, :, mt * 128:(mt + 1) * 128],
                                        pst[:])
                            for h2, mv_t in ((0, mov_a), (1, mov_b)):
                                g = 2 * gp + h2
                                nc.sync.dma_start(
                                    adjT8[:, g * GRP:(g + 1) * GRP, :], mv_t[:])
                                for j in range(0, GRP, 2):
                                    ct = g * GRP + j
                                    for ch in range(NCH):
                                        nc.tensor.matmul(
                                            psy[:, ch * MCH:(ch + 1) * MCH],
                                            xstat[:, ct:ct + 2, :],
                                            mv_t[:, j:j + 2,
                                                 ch * MCH:(ch + 1) * MCH],
                                            start=(ct == 0),
                                            stop=(ct == CT - 2),
                                            perf_mode=mybir.MatmulPerfMode
                                            .DoubleRow)
                else:
                    # rotated schedule: slot s sources group (rot+s) mod NGRP;
                    # own slots (s < S_OWN) read the local xnp pair tiles
                    for s in range(NGRP):
                        gsrc = (rot + s) & (NGRP - 1)
                        if s >= S_OWN:
                            nc.scalar.dma_start(
                                xstat[:, s * SGRP:(s + 1) * SGRP, :],
                                xpf.opt()[
                                    bass.ds(gsrc * (SGRP * 128), SGRP * 128), :]
                                .rearrange("(t p) f -> p t f", p=128))
                        mov = emit_mov_dma(s)
                        for j in range(0, GRP, 2):
                            if s < S_OWN:
                                statT = xnp_prev[:, s * SGRP + j:
                                                 s * SGRP + j + 2, :]
                            else:
                                statT = xstat[:, s * SGRP + j:
                                              s * SGRP + j + 2, :]
                            for ch in range(NCH):
                                nc.tensor.matmul(
                                    psy[:, ch * MCH:(ch + 1) * MCH],
                                    statT,
                                    mov[:, j:j + 2, ch * MCH:(ch + 1) * MCH],
                                    start=(s == 0 and j == 0),
                                    stop=(s == NGRP - 1 and j == GRP - 2),
                                    perf_mode=mybir.MatmulPerfMode.DoubleRow)

                # epilogue: xnewT = C0*(psum_hi + psum_lo) + 0.1*h0^T;
                # hi8 produced straight from psum to shorten the AG chain
                u = pend.tile([H, shard], f32, tag="u")
                nc.vector.scalar_tensor_tensor(
                    out=u[:], in0=psy[0:H, :], scalar=C0, op0=OP.mult,
                    in1=h0s[:], op1=OP.add)
                xnewT = pend.tile([H, shard], f32, tag="xnewT")
                if k < k_steps:
                    hiT = pend.tile([H, shard], fp8, tag="hiT")
                    nc.vector.scalar_tensor_tensor(
                        out=hiT[:], in0=psy[H:128, :], scalar=C0, op0=OP.mult,
                        in1=u[:], op1=OP.add)
                    nc.vector.scalar_tensor_tensor(
                        out=xnewT[:], in0=psy[H:128, :], scalar=C0,
                        op0=OP.mult, in1=u[:], op1=OP.add)
                    loT = pend.tile([H, shard], fp8, tag="loT")
                    nc.vector.tensor_tensor(out=loT[:], in0=xnewT[:],
                                            in1=hiT[:], op=OP.subtract)
                    agin, xnp_prev = step_end_pre(xnewT, hiT, loT)
                    xpf = emit_ag(agin)
                else:
                    nc.vector.scalar_tensor_tensor(
                        out=xnewT[:], in0=psy[H:128, :], scalar=C0,
                        op0=OP.mult, in1=u[:], op1=OP.add)

            # ---------------- head ----------------
            # out2 = x_cl natural [shard, H]
            xcn = pend.tile([128, MT, H], f32, tag="xcn")
            for t in range(MT):
                pp2 = pstep.tile([128, H], f32, tag="sm")
                nc.tensor.transpose(
                    pp2[:], xnewT[:, t * 128:(t + 1) * 128], eye32[0:H, 0:H])
                nc.vector.tensor_copy(xcn[:, t, :], pp2[:])
            nc.sync.dma_start(
                out2_ext.rearrange("(t p) h -> p t h", p=128), xcn[:])

            # out1 = log_softmax(x_cl @ W2) — ops batched by type so the
            # ACT Exp/Ln tables each load once
            hsb = pend.tile([128, MT, NCLASS], f32, tag="hsb")
            negmax = pend.tile([128, MT], f32, tag="negmax")
            for t in range(MT):
                psh2 = pstep.tile([128, NCLASS], f32, tag="sm")
                nc.tensor.matmul(
                    psh2[:], xnewT[:, t * 128:(t + 1) * 128], w2_sb[:],
                    start=True, stop=True)
                nc.vector.tensor_copy(hsb[:, t, :], psh2[:])
                nc.vector.tensor_reduce(
                    negmax[:, t:t + 1], psh2[:], axis=mybir.AxisListType.X,
                    op=OP.max, negate=True)
            esb = pend.tile([128, MT, NCLASS], f32, tag="esb")
            sumexp = pend.tile([128, MT], f32, tag="sumexp")
            for t in range(MT):
                nc.scalar.activation(
                    esb[:, t, :], hsb[:, t, :], AF.Exp,
                    bias=negmax[:, t:t + 1], scale=1.0,
                    accum_out=sumexp[:, t:t + 1])
            lse = pend.tile([128, MT], f32, tag="lse")
            nc.scalar.activation(lse[:], sumexp[:], AF.Ln)
            for t in range(MT):
                o1 = pend.tile([128, NCLASS], f32, tag="o1")
                nc.vector.tensor_scalar(
                    out=o1[:], in0=hsb[:, t, :], scalar1=negmax[:, t:t + 1],
                    scalar2=lse[:, t:t + 1], op0=OP.add, op1=OP.subtract)
                nc.scalar.dma_start(out1_ext[t * 128:(t + 1) * 128, :], o1[:])

    nc.finalize()
    return nc


_NC_CACHE = {}
LAST_RESULT = None


def _get_nc(key=(N, NFEAT, K, NCORES)):
    if key not in _NC_CACHE:
        _NC_CACHE[key] = build(*key)
    return _NC_CACHE[key]


def kernel(x, adj, W1, W2):
    n = adj.shape[0]
    shard = n // NCORES
    nc = _get_nc((n, x.shape[1], K, NCORES))
    x = np.ascontiguousarray(np.asarray(x, dtype=np.float32))
    adj = np.ascontiguousarray(np.asarray(adj, dtype=np.float32))
    W1 = np.ascontiguousarray(np.asarray(W1, dtype=np.float32))
    W2 = np.ascontiguousarray(np.asarray(W2, dtype=np.float32))
    in_maps = [
        {"x": x[i * shard:(i + 1) * shard],
         "adj": adj[i * shard:(i + 1) * shard],
         "W1": W1, "W2": W2}
        for i in range(NCORES)
    ]
    import os
    trace = bool(os.environ.get("KERNEL_TRACE"))
    res = run_bass_kernel_spmd(nc, in_maps, list(range(NCORES)), trace=trace)
    global LAST_RESULT
    LAST_RESULT = res
    out1 = np.concatenate([res.results[i]["out1"] for i in range(NCORES)], axis=0)
    out2 = np.concatenate([res.results[i]["out2"] for i in range(NCORES)], axis=0)
    return out1, out2


# revision 25
# speedup vs baseline: 1.0374x; 1.0374x over previous
"""APPNP forward on 8 TRN2 NeuronCores (Bass/Tile).

Math (reference):
    h0  = relu(x @ W1)                       [N, H]
    x_cl = h0; repeat K: x_cl = 0.9*(adj @ x_cl) + 0.1*h0
    out = (log_softmax(x_cl @ W2, -1), x_cl)

Distribution: row-shard adj/x across 8 cores (SHARD = N/8 rows each).
Per step each core computes its block-row adj_i @ x_cl with x_cl
replicated via AllGather.

Kernel strategy (per core):
  - adj_i is transposed+quantized ONCE on-chip (fused into step 1):
    f32 -> ACT scale by 4096 -> bf16 -> PE transpose -> fp8e4 (stored
    as adjT8 [N, SHARD] in DRAM).  Row-stochastic averaging makes the
    fp8 quantization error negligible (measured ~4e-4 end-to-end).
  - x_cl is carried in an fp8e4 hi/lo split (~9e-4 end-to-end): the
    stationary operand packs [hi | lo] -> psum [128, m] accumulates
    (adj@x)^T in hi/lo halves; epilogue recombines with 0.9/4096 scale
    and adds 0.1*h0.
  - Per step: 256 DoubleRow fp8 matmuls (64 c-tile pairs x 4 m-chunks
    of 512); the moving operand is the fp8 adjT stream in a
    partition-major DRAM layout (16KB-contiguous runs, 32MB/core/step).
  - x_cl^T [64, SHARD] -> hi/lo fp8 -> PE transpose (stride-2 psum) ->
    [SHARD, 128] fp8 pair tiles -> AllGather -> next step's stationary.
  - Final step skips the AllGather and computes the log_softmax head
    locally.
"""

import numpy as np

import concourse.bass as bass
import concourse.bacc as bacc
import concourse.mybir as mybir
import concourse.tile as tile
from concourse.bass_utils import run_bass_kernel_spmd

# Problem constants (hardcoded; the grading harness supplies matching inputs)
N, NFEAT, NHID, NCLASS = 16384, 512, 64, 40
K, ALPHA = 10, 0.1
NCORES = 8

SC = 4096.0              # adj prescale so fp8e4 sees ~[0, 1) values
C0 = (1.0 - ALPHA) / SC  # epilogue scale for psum

dt = mybir.dt
f32, bf16, fp8 = dt.float32, dt.bfloat16, dt.float8e4
AF = mybir.ActivationFunctionType
OP = mybir.AluOpType


def build(n=N, nfeat=NFEAT, k_steps=K, ncores=NCORES):
    shard = n // ncores
    MT = shard // 128          # m-tiles per shard
    CT = n // 128              # c-tiles (contraction)
    MCH = min(512, shard)      # psum chunk (<= 1 bank)
    NCH = shard // MCH         # chunks per shard
    FT = nfeat // 128          # feature tiles
    GRP = 8                    # c-tiles per moving-stream DMA
    NGRP = CT // GRP
    H = NHID

    nc = bacc.Bacc(None)
    x_ext = nc.dram_tensor("x", [shard, nfeat], f32, kind="ExternalInput")
    adj_ext = nc.dram_tensor("adj", [shard, n], f32, kind="ExternalInput")
    w1_ext = nc.dram_tensor("W1", [nfeat, H], f32, kind="ExternalInput")
    w2_ext = nc.dram_tensor("W2", [H, NCLASS], f32, kind="ExternalInput")
    eye_ext = nc.inline_tensor(np.eye(128, dtype=np.float32), "eye128")
    out1_ext = nc.dram_tensor("out1", [shard, NCLASS], f32, kind="ExternalOutput")
    out2_ext = nc.dram_tensor("out2", [shard, H], f32, kind="ExternalOutput")

    with tile.TileContext(nc) as tc:
        with (
            tc.tile_pool(name="const", bufs=1) as constp,
            tc.tile_pool(name="hfam", bufs=1) as hfam,
            tc.tile_pool(name="stat", bufs=1) as statp,
            tc.tile_pool(name="mv", bufs=5) as mvp,
            tc.tile_pool(name="pend", bufs=1) as pend,
            tc.tile_pool(name="psy", bufs=1, space="PSUM") as psyp,
            tc.tile_pool(name="pstep", bufs=2, space="PSUM") as pstep,  # 1-bank small tiles
            tc.tile_pool(name="dram", bufs=3, space="DRAM") as dram,
            tc.tile_pool(name="dram1", bufs=1, space="DRAM") as dram1,
        ):
            # ---------------- constants ----------------
            eye32 = constp.tile([128, 128], f32)
            nc.sync.dma_start(eye32[:], eye_ext[:])
            eyeb = constp.tile([128, 128], bf16)
            nc.vector.tensor_copy(eyeb[:], eye32[:])
            eye8 = constp.tile([128, 128], fp8)
            nc.vector.tensor_copy(eye8[:], eye32[:])
            w1_sb = constp.tile([128, FT, H], f32)
            nc.sync.dma_start(w1_sb[:], w1_ext.rearrange("(t p) h -> p t h", p=128))
            w2_sb = constp.tile([H, NCLASS], f32)
            nc.sync.dma_start(w2_sb[:], w2_ext[:])

            adjT8 = dram1.tile([128, CT, shard], fp8)  # partition-major

            # tiny dummy AllGather to absorb first-collective warmup cost
            wu_in = dram1.tile([128, 16], bf16)
            wu_out = dram1.tile([128 * ncores, 16], bf16, addr_space="Shared")
            wu_sb = constp.tile([128, 16], bf16)
            nc.vector.tensor_copy(wu_sb[:], eye32[:, 0:16])
            nc.sync.dma_start(wu_in.opt(), wu_sb[:])
            nc.gpsimd.collective_compute(
                "AllGather", OP.bypass, replica_groups=[list(range(ncores))],
                ins=[wu_in.opt()], outs=[wu_out.opt()])

            # ---------------- phase 0: h0 = relu(x @ W1) ----------------
            h0s = hfam.tile([H, shard], f32)     # 0.1 * h0^T
            xclT = hfam.tile([H, shard], f32)    # current x_cl^T (f32)
            with tc.tile_pool(name="ph0", bufs=1) as ph0:
                HM = MT // 2 if MT >= 2 else MT  # m-tiles per half
                NHALF = MT // HM
                psh_full = psyp.tile([128, shard], f32, tag="psy")
                psh = psh_full[0:H, :]
                for half in range(NHALF):
                    x_sb = ph0.tile([128, HM, nfeat], f32, tag="x_sb")
                    nc.sync.dma_start(
                        x_sb[:],
                        x_ext[half * HM * 128:(half + 1) * HM * 128, :]
                        .rearrange("(t p) f -> p t f", p=128))
                    xT = ph0.tile([128, FT, HM * 128], f32, tag="xT")
                    for t in range(HM):
                        for ft in range(FT):
                            pxt = pstep.tile([128, 128], f32, tag="sm")
                            nc.tensor.transpose(
                                pxt[:], x_sb[:, t, ft * 128:(ft + 1) * 128],
                                eye32[:])
                            nc.vector.tensor_copy(
                                xT[:, ft, t * 128:(t + 1) * 128], pxt[:])
                    hb = half * HM * 128
                    nchh = (HM * 128) // MCH if HM * 128 >= MCH else 1
                    mchh = min(MCH, HM * 128)
                    for ft in range(FT):
                        for ch in range(nchh):
                            nc.tensor.matmul(
                                psh[:, hb + ch * mchh:hb + (ch + 1) * mchh],
                                w1_sb[:, ft, :],
                                xT[:, ft, ch * mchh:(ch + 1) * mchh],
                                start=(ft == 0), stop=(ft == FT - 1))
                nc.scalar.activation(h0s[:], psh[:], AF.Relu, scale=ALPHA)
                nc.scalar.activation(xclT[:], psh[:], AF.Relu, scale=1.0)

            # step-end: xclT f32 -> hi/lo fp8 -> transpose -> agin (no AG)
            def step_end_pre(xsrc, hiT=None, loT=None):
                if hiT is None:
                    hiT = pend.tile([H, shard], fp8, tag="hiT")
                    loT = pend.tile([H, shard], fp8, tag="loT")
                    nc.vector.tensor_copy(hiT[:], xsrc[:])
                    nc.vector.tensor_tensor(out=loT[:], in0=xsrc[:],
                                            in1=hiT[:], op=OP.subtract)
                xnp = pend.tile([128, MT, 128], fp8, tag="xnp")
                for t in range(MT):
                    pp = pstep.tile([128, 256], fp8, tag="sm")
                    hv = pp[:, 0:128].rearrange(
                        "p (m two) -> p m two", two=2)[:, :, 0]
                    lv = pp[:, 128:256].rearrange(
                        "p (m two) -> p m two", two=2)[:, :, 0]
                    nc.tensor.transpose(
                        hv, hiT[:, t * 128:(t + 1) * 128], eye8[0:H, 0:H])
                    nc.tensor.transpose(
                        lv, loT[:, t * 128:(t + 1) * 128], eye8[0:H, 0:H])
                    src_v = pp[:].rearrange(
                        "p (h m two) -> p h m two", h=2, two=2)[:, :, :, 0]
                    dst_v = xnp[:, t, :].rearrange("p (h m) -> p h m", h=2)
                    nc.vector.tensor_copy(dst_v, src_v)
                agin = dram.tile([shard, 128], fp8, tag="agin")
                nc.scalar.dma_start(
                    agin.opt().rearrange("(t p) f -> p t f", p=128), xnp[:])
                return agin, xnp

            def emit_ag(agin):
                xpf = dram.tile([n, 128], fp8, addr_space="Shared", tag="xpf")
                nc.gpsimd.collective_compute(
                    "AllGather", OP.bypass,
                    replica_groups=[list(range(ncores))],
                    ins=[agin.opt()], outs=[xpf.opt()])
                return xpf

            agin0, xnp_prev = step_end_pre(xclT)
            xpf = emit_ag(agin0)


            # ---------------- K propagation steps ----------------
            SGRP = 8                      # c-tiles per xstat DMA group
            NSG = CT // SGRP
            S_OWN = MT // SGRP            # groups covered by the local shard
            pid = nc.partition_id()
            rot = pid * S_OWN             # per-core group rotation
            PRE = min(2, NGRP)            # mov groups prefetched across the AG
            mov_pre = []                  # tiles DMA'd before the step's AG

            def emit_mov_dma(s):
                gsrc = (rot + s) & (NGRP - 1)
                mv_t = mvp.tile([128, GRP, shard], fp8, tag="mov")
                nc.sync.dma_start(
                    mv_t[:], adjT8[:, bass.ds(gsrc * GRP, GRP), :])
                return mv_t
            for k in range(1, k_steps + 1):
                psy = psyp.tile([128, shard], f32, tag="psy")
                xstat = statp.tile([128, CT, 128], fp8, tag="xstat")

                def load_xstat(sg):
                    nc.scalar.dma_start(
                        xstat[:, sg * SGRP:(sg + 1) * SGRP, :],
                        xpf.opt()[sg * SGRP * 128:(sg + 1) * SGRP * 128, :]
                        .rearrange("(t p) f -> p t f", p=128))

                if k == 1:
                    with (
                        tc.tile_pool(name="pp1", bufs=3) as pp1,
                        tc.tile_pool(name="pp1ps", bufs=2, space="PSUM") as pp1ps,
                    ):
                        for gp in range(NGRP // 2):
                            load_xstat(2 * gp)
                            load_xstat(2 * gp + 1)
                            mov_a = mvp.tile([128, GRP, shard], fp8, tag="mov")
                            mov_b = mvp.tile([128, GRP, shard], fp8, tag="mov")
                            W2G = 2 * GRP * 128
                            for mt in range(MT):
                                asl = pp1.tile([128, W2G], f32, tag="asl", bufs=4)
                                nc.sync.dma_start(
                                    asl[:],
                                    adj_ext[mt * 128:(mt + 1) * 128,
                                            gp * W2G:(gp + 1) * W2G])
                                abf = pp1.tile([128, W2G], bf16, tag="abf")
                                nc.scalar.activation(abf[:], asl[:], AF.Copy,
                                                     scale=SC)
                                for h2, mv_t in ((0, mov_a), (1, mov_b)):
                                    pst = pp1ps.tile([128, GRP, 128], bf16,
                                                     tag="pst")
                                    for j in range(GRP):
                                        nc.tensor.transpose(
                                            pst[:, j, :],
                                            abf[:, (h2 * GRP + j) * 128:
                                                (h2 * GRP + j + 1) * 128],
                                            eyeb[:])
                                    nc.vector.tensor_copy(
                                        mv_t[:, :, mt * 128:(mt + 1) * 128],
                                        pst[:])
                            for h2, mv_t in ((0, mov_a), (1, mov_b)):
                                g = 2 * gp + h2
                                nc.sync.dma_start(
                                    adjT8[:, g * GRP:(g + 1) * GRP, :], mv_t[:])
                                for j in range(0, GRP, 2):
                                    ct = g * GRP + j
                                    for ch in range(NCH):
                                        nc.tensor.matmul(
                                            psy[:, ch * MCH:(ch + 1) * MCH],
                                            xstat[:, ct:ct + 2, :],
                                            mv_t[:, j:j + 2,
                                                 ch * MCH:(ch + 1) * MCH],
                                            start=(ct == 0),
                                            stop=(ct == CT - 2),
                                            perf_mode=mybir.MatmulPerfMode
                                            .DoubleRow)
                else:
                    # rotated schedule: slot s sources group (rot+s) mod NGRP;
                    # own slots (s < S_OWN) read the local xnp pair tiles
                    for s in range(NGRP):
                        gsrc = (rot + s) & (NGRP - 1)
                        if s >= S_OWN:
                            nc.scalar.dma_start(
                                xstat[:, s * SGRP:(s + 1) * SGRP, :],
                                xpf.opt()[
                                    bass.ds(gsrc * (SGRP * 128), SGRP * 128), :]
                                .rearrange("(t p) f -> p t f", p=128))
                        mov = emit_mov_dma(s)
                        for j in range(0, GRP, 2):
                            if s < S_OWN:
                                statT = xnp_prev[:, s * SGRP + j:
                                                 s * SGRP + j + 2, :]
                            else:
                                statT = xstat[:, s * SGRP + j:
                                              s * SGRP + j + 2, :]
                            for ch in range(NCH):
                                nc.tensor.matmul(
                                    psy[:, ch * MCH:(ch + 1) * MCH],
                                    statT,
                                    mov[:, j:j + 2, ch * MCH:(ch + 1) * MCH],
                                    start=(s == 0 and j == 0),
                                    stop=(s == NGRP - 1 and j == GRP - 2),
                                    perf_mode=mybir.MatmulPerfMode.DoubleRow)

                # epilogue: xnewT = C0*(psum_hi + psum_lo) + 0.1*h0^T;
                # hi8 produced straight from psum to shorten the AG chain
                u = pend.tile([H, shard], f32, tag="u")
                nc.vector.scalar_tensor_tensor(
                    out=u[:], in0=psy[0:H, :], scalar=C0, op0=OP.mult,
                    in1=h0s[:], op1=OP.add)
                xnewT = pend.tile([H, shard], f32, tag="xnewT")
                if k < k_steps:
                    hiT = pend.tile([H, shard], fp8, tag="hiT")
                    nc.vector.scalar_tensor_tensor(
                        out=hiT[:], in0=psy[H:128, :], scalar=C0, op0=OP.mult,
                        in1=u[:], op1=OP.add)
                    nc.vector.scalar_tensor_tensor(
                        out=xnewT[:], in0=psy[H:128, :], scalar=C0,
                        op0=OP.mult, in1=u[:], op1=OP.add)
                    loT = pend.tile([H, shard], fp8, tag="loT")
                    nc.vector.tensor_tensor(out=loT[:], in0=xnewT[:],
                                            in1=hiT[:], op=OP.subtract)
                    agin, xnp_prev = step_end_pre(xnewT, hiT, loT)
                    xpf = emit_ag(agin)
                else:
                    nc.vector.scalar_tensor_tensor(
                        out=xnewT[:], in0=psy[H:128, :], scalar=C0,
                        op0=OP.mult, in1=u[:], op1=OP.add)

            # ---------------- head ----------------
            # out2 = x_cl natural [shard, H]
            xcn = pend.tile([128, MT, H], f32, tag="xcn")
            for t in range(MT):
                pp2 = pstep.tile([128, H], f32, tag="sm")
                nc.tensor.transpose(
                    pp2[:], xnewT[:, t * 128:(t + 1) * 128], eye32[0:H, 0:H])
                nc.vector.tensor_copy(xcn[:, t, :], pp2[:])
            nc.sync.dma_start(
                out2_ext.rearrange("(t p) h -> p t h", p=128), xcn[:])

            # out1 = log_softmax(x_cl @ W2) — ops batched by type so the
            # ACT Exp/Ln tables each load once
            hsb = pend.tile([128, MT, NCLASS], f32, tag="hsb")
            negmax = pend.tile([128, MT], f32, tag="negmax")
            for t in range(MT):
                psh2 = pstep.tile([128, NCLASS], f32, tag="sm")
                nc.tensor.matmul(
                    psh2[:], xnewT[:, t * 128:(t + 1) * 128], w2_sb[:],
                    start=True, stop=True)
                nc.vector.tensor_copy(hsb[:, t, :], psh2[:])
                nc.vector.tensor_reduce(
                    negmax[:, t:t + 1], psh2[:], axis=mybir.AxisListType.X,
                    op=OP.max, negate=True)
            esb = pend.tile([128, MT, NCLASS], f32, tag="esb")
            sumexp = pend.tile([128, MT], f32, tag="sumexp")
            for t in range(MT):
                nc.scalar.activation(
                    esb[:, t, :], hsb[:, t, :], AF.Exp,
                    bias=negmax[:, t:t + 1], scale=1.0,
                    accum_out=sumexp[:, t:t + 1])
            lse = pend.tile([128, MT], f32, tag="lse")
            nc.scalar.activation(lse[:], sumexp[:], AF.Ln)
            for t in range(MT):
                o1 = pend.tile([128, NCLASS], f32, tag="o1")
                nc.vector.tensor_scalar(
                    out=o1[:], in0=hsb[:, t, :], scalar1=negmax[:, t:t + 1],
                    scalar2=lse[:, t:t + 1], op0=OP.add, op1=OP.subtract)
                nc.scalar.dma_start(out1_ext[t * 128:(t + 1) * 128, :], o1[:])

    nc.finalize()
    return nc


_NC_CACHE = {}
LAST_RESULT = None


def _get_nc(key=(N, NFEAT, K, NCORES)):
    if key not in _NC_CACHE:
        _NC_CACHE[key] = build(*key)
    return _NC_CACHE[key]


def kernel(x, adj, W1, W2):
    n = adj.shape[0]
    shard = n // NCORES
    nc = _get_nc((n, x.shape[1], K, NCORES))
    x = np.ascontiguousarray(np.asarray(x, dtype=np.float32))
    adj = np.ascontiguousarray(np.asarray(adj, dtype=np.float32))
    W1 = np.ascontiguousarray(np.asarray(W1, dtype=np.float32))
    W2 = np.ascontiguousarray(np.asarray(W2, dtype=np.float32))
    in_maps = [
        {"x": x[i * shard:(i + 1) * shard],
         "adj": adj[i * shard:(i + 1) * shard],
         "W1": W1, "W2": W2}
        for i in range(NCORES)
    ]
    import os
    trace = bool(os.environ.get("KERNEL_TRACE"))
    res = run_bass_kernel_spmd(nc, in_maps, list(range(NCORES)), trace=trace)
    global LAST_RESULT
    LAST_RESULT = res
    out1 = np.concatenate([res.results[i]["out1"] for i in range(NCORES)], axis=0)
    out2 = np.concatenate([res.results[i]["out2"] for i in range(NCORES)], axis=0)
    return out1, out2
